# revision 36
# baseline (speedup 1.0000x reference)
"""Trainium2 Bass kernel for nn_MemoryReader (retrieval_knn).

Math (per batch b):
  mk_h [h,c,n] (c=16, n=THW=8192), qk_h/qe_h [h,c,m] (m=HW=1024)
  logits[h,n,m] = (ms[n]/8) * ( sum_c mk^3*(-qe) + mk*(2*qk*qe) + (-b_sq) )
  aff = softmax over h
  mem[h,c',m] = sum_n mo[h,c',n] * aff[h,n,m]   (c'=128)
  out = concat(mem, qv)

Sharding: 8 cores = 2 batches x 4 THW-chunks (n-chunk 2048/core). Softmax is
over heads -> core-local. Readout partial-sums over n are reduced on host
during the gather (legit unshard of a contraction-sharded axis).

Device kernel per core (v2):
  32 iterations (mh in 2 m-halves x nt in 16 n-tiles of 128).
  Per iteration: 2 pair-sims (heads {2p,2p+1} as two concurrent K=64
  matmuls via tile_position) -> per-pair EXP (ACT, PSUM->SBUF bf16).
  Softmax tail batched over DVE_BATCH iterations to amortize DVE init
  overhead: one add (pair-halves), Pool add (fold to S), custom NR
  reciprocal, one broadcast multiply -> aff (bf16). Readout matmuls
  (bf16, K=128) accumulate over nt in PSUM.
  All matmul operands bf16 (host-precomputed x/w rows; |logit| small and
  the readout averages over 2048 n so rounding noise cancels).
"""

import sys

sys.path.insert(0, "/opt/trn_rl_repo")

import numpy as np

import concourse.bass as bass
import concourse.tile as tile
from concourse import bacc, mybir
from concourse.bass_utils import run_bass_kernel_spmd

try:
    import ml_dtypes

    _BF16_NP = np.dtype(ml_dtypes.bfloat16)
except ImportError:  # pragma: no cover
    _BF16_NP = None

HEADS, B, CK, CV = 4, 2, 64, 512
T, H, W = 8, 32, 32
THW, HW = T * H * W, H * W          # 8192, 1024
C = CK // HEADS                      # 16
NCHUNK = THW // 4                    # 2048 n per core
NT = NCHUNK // 128                   # 16 n-tiles per core
KDIM = 2 * C + 1                     # 33

F32 = mybir.dt.float32
F32R = mybir.dt.float32r
BF16 = mybir.dt.bfloat16

# ---- tunables -------------------------------------------------------------
SIM_DT = BF16            # sim matmul operand dtype (BF16 | F32R)
DVE_BATCH = 4            # iterations batched per DVE softmax-tail op group
HEATER_N = 2             # dummy warm-up matmuls
                         # ramp ignores them; they only delay the first sims
RO_PRIORITY = 10000000       # readout matmul bass_priority (None = emission order)
# ---------------------------------------------------------------------------


def _np_dt(dt):
    return _BF16_NP if dt == BF16 else np.float32


def build_bass():
    nc = bacc.Bacc(None)
    sim_dt = SIM_DT
    PB = NCHUNK + HW  # per-pair free block: [X 2048 | W 1024]
    xw_d = nc.dram_tensor("xw", [128, 2 * PB], sim_dt, kind="ExternalInput")
    mvt_d = nc.dram_tensor("mvt", [NCHUNK, CV], BF16, kind="ExternalInput")
    mem_d = nc.dram_tensor("mem", [CV, HW], F32, kind="ExternalOutput")

    Exp = mybir.ActivationFunctionType.Exp
    Copy = mybir.ActivationFunctionType.Copy

    from concourse.dve_ops import (
        RECIP_APPROX_FAST_CONSTS as _RC,
        RECIPROCAL_APPROX_FAST as _RF,
    )

    GW = DVE_BATCH * 2048  # e/aff group width (4 heads x 512 m per iter)

    with tile.TileContext(nc) as tc:
        with (
            tc.tile_pool(name="const", bufs=1) as constp,
            tc.tile_pool(name="simp", bufs=2, space="PSUM") as simp,
            tc.tile_pool(name="memp", bufs=1, space="PSUM") as memp,
            tc.tile_pool(name="ework", bufs=2) as ework,
            tc.tile_pool(name="outp", bufs=2) as outp,
        ):
            xw_sb = constp.tile([128, 2 * PB], sim_dt)
            mvt_sb = constp.tile([128, NT * CV], BF16)

            # DMA-in, ordered so the earliest-needed chunks land first:
            # W(mh0) for both pairs, then X quarters interleaved with mvt
            # n-tiles (mvt[q] needed once iteration q*4's softmax drains),
            # W(mh1) last (first needed at iteration 16).
            def dma_w(pr, mh, eng=None):
                o = pr * PB + NCHUNK + mh * 512
                (eng or nc.sync).dma_start(
                    out=xw_sb[:, o : o + 512], in_=xw_d[:, o : o + 512]
                )

            def dma_x(pr, q, eng=None, w=512):
                o = pr * PB + q * 512
                (eng or nc.sync).dma_start(
                    out=xw_sb[:, o : o + w], in_=xw_d[:, o : o + w]
                )

            def dma_mvt(q):
                # one DMA per 128-row nt-tile (4 tiles per call)
                for nt in range(4 * q, 4 * q + 4):
                    nc.sync.dma_start(
                        out=mvt_sb[:, nt * CV : (nt + 1) * CV],
                        in_=mvt_d[nt * 128 : (nt + 1) * 128, :],
                    )

            # first-needed chunks spread over three DMA-capable queues
            # (SP/Activation/GpSimd) so their transfers run in parallel (one
            # queue serializes at ~1.3us per 128KB chunk and the first sims
            # wait on w00+w10+x-heads)
            dma_w(0, 0, eng=nc.sync)
            dma_w(1, 0, eng=nc.scalar)
            dma_x(0, 0, eng=nc.gpsimd, w=128)   # nt0 slice only
            dma_x(1, 0, eng=nc.gpsimd, w=128)
            nc.sync.dma_start(out=xw_sb[:, 128:512], in_=xw_d[:, 128:512])
            nc.scalar.dma_start(
                out=xw_sb[:, PB + 128 : PB + 512], in_=xw_d[:, PB + 128 : PB + 512]
            )
            dma_mvt(0)
            dma_x(0, 1)
            dma_x(1, 1)
            dma_mvt(1)
            dma_x(0, 2)
            dma_x(1, 2)
            dma_mvt(2)
            dma_x(0, 3)
            dma_x(1, 3)
            dma_mvt(3)
            dma_w(0, 1)
            dma_w(1, 1)

            # Heater: back-to-back dummy MMs start the PE p-state ramp while
            # the input DMAs stream.
            hsrc = constp.tile([64, 768], BF16)
            nc.vector.memset(hsrc[:], 0.0)
            warm = simp.tile([128, 1024], F32, tag="sim")
            for _ in range(HEATER_N):
                wmm = nc.tensor.matmul(
                    warm[:, :512],
                    lhsT=hsrc[:, 0:128],
                    rhs=hsrc[:, 128:640],
                    start=True,
                    stop=True,
                    tile_position=(0, 0),
                )
                wmm.ins.bass_priority = -100

            iters = [(mh, nt) for mh in range(2) for nt in range(NT)]
            # progressively smaller batches at the end: the batched-chain
            # latency (add1 -> Pool -> recip -> mul, ~13us at batch 4) has no
            # exp work left to hide under after the final sims, so the tail
            # runs small low-latency chains instead.
            batch_sizes = [DVE_BATCH] * 6 + [2, 2] + [1] * 4
            assert sum(batch_sizes) == len(iters)
            mem_ps = None
            pending_ro = []  # previous group's readouts: (mh, nt, aff_b, j)

            def emit_ro_heads(ro, heads):
                # Emit a subset of one iteration's readout matmuls. Allocating
                # mem_ps happens with the first head of nt==0; the flush fires
                # after the last head of nt==15.
                nonlocal mem_ps
                mh, nt, aff_s, j = ro
                if nt == 0 and heads[0] == 0:
                    mem_ps = memp.tile([128, 4 * 512], F32)
                for h in heads:
                    rom = nc.tensor.matmul(
                        mem_ps[:, h * 512 : (h + 1) * 512],
                        lhsT=mvt_sb[:, nt * CV + h * 128 : nt * CV + h * 128 + 128],
                        rhs=aff_s[:, j * 2048 + h * 512 : j * 2048 + h * 512 + 512],
                        start=(nt == 0),
                        stop=(nt == NT - 1),
                    )
                    if RO_PRIORITY is not None:
                        rom.ins.bass_priority = RO_PRIORITY
                if nt == NT - 1 and heads[-1] == HEADS - 1:
                    # flush this m-half: PSUM -> SBUF staging -> DRAM. Copies
                    # split ACT/DVE (both near their walls mid-kernel); DMAs
                    # alternate the SP and GpSimd queues so the four 256KB
                    # transfers drain in parallel.
                    mem_sb = outp.tile([128, 4 * 512], F32)
                    for h in range(HEADS):
                        src = mem_ps[:, h * 512 : (h + 1) * 512]
                        dst = mem_sb[:, h * 512 : (h + 1) * 512]
                        # mh0's flush lands mid-sprint: keep its copies off
                        # the ACT critical path (exps). mh1's flush is in the
                        # tail where ACT is free — split it with DVE.
                        if mh == 0 or h >= 2:
                            cp = nc.vector.tensor_copy(dst, src)
                        else:
                            cp = nc.scalar.activation(dst, src, Copy)
                        cp.ins.bass_priority = 45
                        dma_eng = nc.sync if h % 2 == 0 else nc.gpsimd
                        dma_eng.dma_start(
                            out=mem_d[h * 128 : (h + 1) * 128,
                                      mh * 512 : (mh + 1) * 512],
                            in_=dst,
                        )

            def emit_readouts(ro_list):
                for ro in ro_list:
                    emit_ro_heads(ro, [0, 1, 2, 3])

            it0 = 0
            for g, nb_g in enumerate(batch_sizes):
                group = iters[it0 : it0 + nb_g]
                it0 += nb_g
                # e_b bufs=3: the batched DVE chain's serial latency (~13us
                # add1->Pool->recip->mul) exceeds the ~9us group period, so
                # with 2 bufs the first exp of group g+2 WAR-stalls on mul(g).
                e_b = ework.tile([128, GW], BF16, tag="e", bufs=5)
                for j, (mh, nt) in enumerate(group):
                    for pr in range(2):
                        ps = simp.tile([128, 1024], F32, tag="sim")
                        for half in range(2):
                            base = half * 64
                            nc.tensor.matmul(
                                ps[:, half * 512 : half * 512 + 512],
                                lhsT=xw_sb[base : base + 64,
                                           pr * PB + nt * 128 : pr * PB + nt * 128 + 128],
                                rhs=xw_sb[base : base + 64,
                                          pr * PB + NCHUNK + mh * 512 : pr * PB + NCHUNK + mh * 512 + 512],
                                start=True,
                                stop=True,
                                tile_position=(base, 0),
                            )
                        eo = j * 2048 + pr * 1024
                        nc.scalar.activation(e_b[:, eo : eo + 1024], ps[:], Exp)
                        # interleave previous-group readouts at 2-matmul
                        # granularity after each pair: long readout runs in
                        # the in-order PE queue block the next sims (which
                        # gate the exps) for up to ~1.3us during throttled
                        # phases. tile_wait_until floors them at this
                        # iteration's real-time position in the scheduler's
                        # timeline so its (optimistic, full-clock) PE model
                        # can't pack them earlier into dense runs.
                        if pending_ro:
                            cur_it = it0 - nb_g + j
                            with tc.tile_wait_until((6000 + cur_it * 2100) / 1e6):
                                emit_ro_heads(
                                    pending_ro[0], [0, 1] if pr == 0 else [2, 3]
                                )
                            if pr == 1:
                                del pending_ro[:1]

                # ---- batched softmax tail ----
                nb = len(group)
                # sp[j, x] = e[j, pairA x] + e[j, pairB x]  (x = 2 heads x 512)
                sp_b = ework.tile([128, DVE_BATCH * 1024], BF16, tag="sp")
                e3 = e_b[:, : nb * 2048].rearrange("p (j x) -> p j x", j=nb)
                nc.vector.tensor_add(
                    sp_b[:, : nb * 1024].rearrange("p (j x) -> p j x", j=nb),
                    e3[:, :, 0:1024],
                    e3[:, :, 1024:2048],
                )
                # s[j, m] = sp[j, m] + sp[j, m+512]  (fold remaining 2 heads)
                # Pool for the big steady-state groups (off the DVE), DVE for
                # the small tail groups (Pool's dispatch+contention latency
                # would sit mid-chain in the un-hidden tail).
                s_b = ework.tile([128, DVE_BATCH * 512], F32, tag="S")
                sp3 = sp_b[:, : nb * 1024].rearrange("p (j x) -> p j x", j=nb)
                add2_eng = nc.gpsimd if nb > 2 else nc.vector
                add2_eng.tensor_add(
                    s_b[:, : nb * 512].rearrange("p (j x) -> p j x", j=nb),
                    sp3[:, :, 0:512],
                    sp3[:, :, 512:1024],
                )
                # r = 1/s (fast NR approx, bf16 out at the DVE write port)
                r_b = ework.tile([128, DVE_BATCH * 512], BF16, tag="R")
                nc.vector._custom_dve(
                    _RF,
                    out=r_b[:, : nb * 512],
                    in0=s_b[:, : nb * 512],
                    s0=_RC["s0"],
                    s1=_RC["s1"],
                    imm2=_RC["imm2"],
                )
                # aff[j, h, m] = e[j, h, m] * r[j, m]
                aff_b = ework.tile([128, DVE_BATCH * 2048], BF16, tag="aff")
                if nb > 2:
                    r3 = r_b[:, : nb * 512].rearrange("p (j m) -> p j m", j=nb)
                    nc.vector.tensor_mul(
                        aff_b[:, : nb * 2048].rearrange(
                            "p (j h m) -> p j h m", j=nb, h=4
                        ),
                        e_b[:, : nb * 2048].rearrange(
                            "p (j h m) -> p j h m", j=nb, h=4
                        ),
                        r3[:, :, None, :].to_broadcast((128, nb, 4, 512)),
                    )
                else:
                    # tail: per-iteration muls so each iteration's readouts
                    # can start without waiting for the whole batch
                    for j in range(nb):
                        nc.vector.tensor_mul(
                            aff_b[:, j * 2048 : (j + 1) * 2048].rearrange(
                                "p (h m) -> p h m", h=4
                            ),
                            e_b[:, j * 2048 : (j + 1) * 2048].rearrange(
                                "p (h m) -> p h m", h=4
                            ),
                            r_b[:, j * 512 : (j + 1) * 512][
                                :, None, :
                            ].to_broadcast((128, 4, 512)),
                        )

                # queue this group's readouts; emitted interleaved with the
                # next group's sims (or drained after the loop).
                pending_ro.extend((mh, nt, aff_b, j) for j, (mh, nt) in enumerate(group))
            emit_readouts(pending_ro)
    return nc


def host_decompose(mk, qk, ms, qe, mv):
    """Build the 8 per-core input dicts."""
    mk_f = np.asarray(mk, np.float32).reshape(B, CK, THW)
    mv_f = np.asarray(mv, np.float32).reshape(B, CV, THW)
    ms_f = np.asarray(ms, np.float32).reshape(B, THW)
    qk_h = np.asarray(qk, np.float32).reshape(B, HEADS, C, HW)
    qe_h = np.asarray(qe, np.float32).reshape(B, HEADS, C, HW)

    msn = ms_f / np.float32(np.sqrt(CK))                       # [B, THW]
    mk3 = mk_f * mk_f * mk_f                                   # [B, CK, THW]

    # w [B, 33, h, m]
    w_all = np.empty((B, KDIM, HEADS, HW), np.float32)
    w_all[:, :C] = -np.swapaxes(qe_h, 1, 2)
    w_all[:, C : 2 * C] = np.swapaxes(2.0 * qk_h * qe_h, 1, 2)
    w_all[:, 2 * C] = -np.sum(qe_h * qk_h**3, axis=2)

    # x [B, 33, h, n]
    x_all = np.empty((B, KDIM, HEADS, THW), np.float32)
    mk3_h = mk3.reshape(B, HEADS, C, THW)
    mk_h = mk_f.reshape(B, HEADS, C, THW)
    x_all[:, :C] = np.swapaxes(mk3_h, 1, 2) * msn[:, None, None, :]
    x_all[:, C : 2 * C] = np.swapaxes(mk_h, 1, 2) * msn[:, None, None, :]
    x_all[:, 2 * C] = msn[:, None, :]

    xw_np = _np_dt(SIM_DT)
    PB = NCHUNK + HW
    in_maps = []
    for core in range(8):
        b, j = core // 4, core % 4
        sl = slice(j * NCHUNK, (j + 1) * NCHUNK)
        xw = np.zeros((128, 2 * PB), np.float32)
        for pr in range(2):
            for half in range(2):
                h = 2 * pr + half
                r0 = half * 64
                xw[r0 : r0 + KDIM, pr * PB : pr * PB + NCHUNK] = x_all[b, :, h, sl]
                xw[r0 : r0 + KDIM, pr * PB + NCHUNK : (pr + 1) * PB] = w_all[b, :, h]
        mvt = np.ascontiguousarray(mv_f[b, :, sl].T).astype(_BF16_NP)
        in_maps.append({"xw": xw.astype(xw_np), "mvt": mvt})
    return in_maps


_NC_CACHE = None


def _get_nc():
    global _NC_CACHE
    if _NC_CACHE is None:
        nc = build_bass()
        if not nc.is_finalized():
            nc.finalize()  # Bacc compile: wait legalization etc.
        _NC_CACHE = nc
    return _NC_CACHE


def kernel(mk, qk, ms, qe, mv, qv, _trace=False, _trace_kwargs=None):
    in_maps = host_decompose(mk, qk, ms, qe, mv)
    nc = _get_nc()
    res = run_bass_kernel_spmd(
        nc, in_maps, list(range(8)), trace=_trace, **(_trace_kwargs or {})
    )
    mem = np.zeros((B, CV, HW), np.float32)
    for core in range(8):
        mem[core // 4] += res.results[core]["mem"]
    out = np.concatenate(
        [mem.reshape(B, CV, H, W), np.asarray(qv, np.float32).reshape(B, CV, H, W)],
        axis=1,
    )
    if _trace:
        return out, res
    return out


# revision 37
# speedup vs baseline: 1.0029x; 1.0029x over previous
"""Trainium2 Bass kernel for nn_MemoryReader (retrieval_knn).

Math (per batch b):
  mk_h [h,c,n] (c=16, n=THW=8192), qk_h/qe_h [h,c,m] (m=HW=1024)
  logits[h,n,m] = (ms[n]/8) * ( sum_c mk^3*(-qe) + mk*(2*qk*qe) + (-b_sq) )
  aff = softmax over h
  mem[h,c',m] = sum_n mo[h,c',n] * aff[h,n,m]   (c'=128)
  out = concat(mem, qv)

Sharding: 8 cores = 2 batches x 4 THW-chunks (n-chunk 2048/core). Softmax is
over heads -> core-local. Readout partial-sums over n are reduced on host
during the gather (legit unshard of a contraction-sharded axis).

Device kernel per core (v2):
  32 iterations (mh in 2 m-halves x nt in 16 n-tiles of 128).
  Per iteration: 2 pair-sims (heads {2p,2p+1} as two concurrent K=64
  matmuls via tile_position) -> per-pair EXP (ACT, PSUM->SBUF bf16).
  Softmax tail batched over DVE_BATCH iterations to amortize DVE init
  overhead: one add (pair-halves), Pool add (fold to S), custom NR
  reciprocal, one broadcast multiply -> aff (bf16). Readout matmuls
  (bf16, K=128) accumulate over nt in PSUM.
  All matmul operands bf16 (host-precomputed x/w rows; |logit| small and
  the readout averages over 2048 n so rounding noise cancels).
"""

import sys

sys.path.insert(0, "/opt/trn_rl_repo")

import numpy as np

import concourse.bass as bass
import concourse.tile as tile
from concourse import bacc, mybir
from concourse.bass_utils import run_bass_kernel_spmd

try:
    import ml_dtypes

    _BF16_NP = np.dtype(ml_dtypes.bfloat16)
except ImportError:  # pragma: no cover
    _BF16_NP = None

HEADS, B, CK, CV = 4, 2, 64, 512
T, H, W = 8, 32, 32
THW, HW = T * H * W, H * W          # 8192, 1024
C = CK // HEADS                      # 16
NCHUNK = THW // 4                    # 2048 n per core
NT = NCHUNK // 128                   # 16 n-tiles per core
KDIM = 2 * C + 1                     # 33

F32 = mybir.dt.float32
F32R = mybir.dt.float32r
BF16 = mybir.dt.bfloat16

# ---- tunables -------------------------------------------------------------
SIM_DT = BF16            # sim matmul operand dtype (BF16 | F32R)
DVE_BATCH = 4            # iterations batched per DVE softmax-tail op group
HEATER_N = 2             # dummy warm-up matmuls
                         # ramp ignores them; they only delay the first sims
RO_PRIORITY = 10000000       # readout matmul bass_priority (None = emission order)
# ---------------------------------------------------------------------------


def _np_dt(dt):
    return _BF16_NP if dt == BF16 else np.float32


def build_bass():
    nc = bacc.Bacc(None)
    sim_dt = SIM_DT
    PB = NCHUNK + HW  # per-pair free block: [X 2048 | W 1024]
    xw_d = nc.dram_tensor("xw", [128, 2 * PB], sim_dt, kind="ExternalInput")
    mvt_d = nc.dram_tensor("mvt", [NCHUNK, CV], BF16, kind="ExternalInput")
    mem_d = nc.dram_tensor("mem", [CV, HW], F32, kind="ExternalOutput")

    Exp = mybir.ActivationFunctionType.Exp
    Copy = mybir.ActivationFunctionType.Copy

    from concourse.dve_ops import (
        RECIP_APPROX_FAST_CONSTS as _RC,
        RECIPROCAL_APPROX_FAST as _RF,
    )

    GW = DVE_BATCH * 2048  # e/aff group width (4 heads x 512 m per iter)

    with tile.TileContext(nc) as tc:
        with (
            tc.tile_pool(name="const", bufs=1) as constp,
            tc.tile_pool(name="simp", bufs=2, space="PSUM") as simp,
            tc.tile_pool(name="memp", bufs=1, space="PSUM") as memp,
            tc.tile_pool(name="ework", bufs=2) as ework,
            tc.tile_pool(name="outp", bufs=2) as outp,
        ):
            xw_sb = constp.tile([128, 2 * PB], sim_dt)
            mvt_sb = constp.tile([128, NT * CV], BF16)

            # DMA-in, ordered so the earliest-needed chunks land first:
            # W(mh0) for both pairs, then X quarters interleaved with mvt
            # n-tiles (mvt[q] needed once iteration q*4's softmax drains),
            # W(mh1) last (first needed at iteration 16).
            def dma_w(pr, mh, eng=None):
                o = pr * PB + NCHUNK + mh * 512
                (eng or nc.sync).dma_start(
                    out=xw_sb[:, o : o + 512], in_=xw_d[:, o : o + 512]
                )

            def dma_x(pr, q, eng=None, w=512):
                o = pr * PB + q * 512
                (eng or nc.sync).dma_start(
                    out=xw_sb[:, o : o + w], in_=xw_d[:, o : o + w]
                )

            def dma_mvt(q):
                # one DMA per 128-row nt-tile (4 tiles per call)
                for nt in range(4 * q, 4 * q + 4):
                    nc.sync.dma_start(
                        out=mvt_sb[:, nt * CV : (nt + 1) * CV],
                        in_=mvt_d[nt * 128 : (nt + 1) * 128, :],
                    )

            # first-needed chunks spread over three DMA-capable queues
            # (SP/Activation/GpSimd) so their transfers run in parallel (one
            # queue serializes at ~1.3us per 128KB chunk and the first sims
            # wait on w00+w10+x-heads)
            dma_w(0, 0, eng=nc.sync)
            dma_w(1, 0, eng=nc.scalar)
            dma_x(0, 0, eng=nc.gpsimd, w=128)   # nt0 slice only
            dma_x(1, 0, eng=nc.gpsimd, w=128)
            nc.sync.dma_start(out=xw_sb[:, 128:512], in_=xw_d[:, 128:512])
            nc.scalar.dma_start(
                out=xw_sb[:, PB + 128 : PB + 512], in_=xw_d[:, PB + 128 : PB + 512]
            )
            dma_mvt(0)
            dma_x(0, 1)
            dma_x(1, 1)
            dma_mvt(1)
            dma_x(0, 2)
            dma_x(1, 2)
            dma_mvt(2)
            dma_x(0, 3)
            dma_x(1, 3)
            dma_mvt(3)
            dma_w(0, 1)
            dma_w(1, 1)

            # Heater: back-to-back dummy MMs start the PE p-state ramp while
            # the input DMAs stream.
            hsrc = constp.tile([64, 768], BF16)
            nc.vector.memset(hsrc[:], 0.0)
            warm = simp.tile([128, 1024], F32, tag="sim")
            for _ in range(HEATER_N):
                wmm = nc.tensor.matmul(
                    warm[:, :512],
                    lhsT=hsrc[:, 0:128],
                    rhs=hsrc[:, 128:640],
                    start=True,
                    stop=True,
                    tile_position=(0, 0),
                )
                wmm.ins.bass_priority = -100

            iters = [(mh, nt) for mh in range(2) for nt in range(NT)]
            # progressively smaller batches at the end: the batched-chain
            # latency (add1 -> Pool -> recip -> mul, ~13us at batch 4) has no
            # exp work left to hide under after the final sims, so the tail
            # runs small low-latency chains instead.
            batch_sizes = [DVE_BATCH] * 6 + [2, 2] + [1] * 4
            assert sum(batch_sizes) == len(iters)
            mem_ps = None
            pending_ro = []  # previous group's readouts: (mh, nt, aff_b, j)

            def emit_ro_heads(ro, heads):
                # Emit a subset of one iteration's readout matmuls. Allocating
                # mem_ps happens with the first head of nt==0; the flush fires
                # after the last head of nt==15.
                nonlocal mem_ps
                mh, nt, aff_s, j = ro
                if nt == 0 and heads[0] == 0:
                    mem_ps = memp.tile([128, 4 * 512], F32)
                for h in heads:
                    rom = nc.tensor.matmul(
                        mem_ps[:, h * 512 : (h + 1) * 512],
                        lhsT=mvt_sb[:, nt * CV + h * 128 : nt * CV + h * 128 + 128],
                        rhs=aff_s[:, j * 2048 + h * 512 : j * 2048 + h * 512 + 512],
                        start=(nt == 0),
                        stop=(nt == NT - 1),
                    )
                    if RO_PRIORITY is not None:
                        rom.ins.bass_priority = RO_PRIORITY
                if nt == NT - 1 and heads[-1] == HEADS - 1:
                    # flush this m-half: PSUM -> SBUF staging -> DRAM. Copies
                    # split ACT/DVE (both near their walls mid-kernel); DMAs
                    # alternate the SP and GpSimd queues so the four 256KB
                    # transfers drain in parallel.
                    mem_sb = outp.tile([128, 4 * 512], F32)
                    for h in range(HEADS):
                        src = mem_ps[:, h * 512 : (h + 1) * 512]
                        dst = mem_sb[:, h * 512 : (h + 1) * 512]
                        if h >= 2:
                            cp = nc.vector.tensor_copy(dst, src)
                        else:
                            cp = nc.scalar.activation(dst, src, Copy)
                        cp.ins.bass_priority = 45
                        dma_eng = nc.sync if h % 2 == 0 else nc.gpsimd
                        dma_eng.dma_start(
                            out=mem_d[h * 128 : (h + 1) * 128,
                                      mh * 512 : (mh + 1) * 512],
                            in_=dst,
                        )

            def emit_readouts(ro_list):
                for ro in ro_list:
                    emit_ro_heads(ro, [0, 1, 2, 3])

            it0 = 0
            for g, nb_g in enumerate(batch_sizes):
                group = iters[it0 : it0 + nb_g]
                it0 += nb_g
                # e_b bufs=3: the batched DVE chain's serial latency (~13us
                # add1->Pool->recip->mul) exceeds the ~9us group period, so
                # with 2 bufs the first exp of group g+2 WAR-stalls on mul(g).
                e_b = ework.tile([128, GW], BF16, tag="e", bufs=4)
                for j, (mh, nt) in enumerate(group):
                    for pr in range(2):
                        ps = simp.tile([128, 1024], F32, tag="sim")
                        for half in range(2):
                            base = half * 64
                            nc.tensor.matmul(
                                ps[:, half * 512 : half * 512 + 512],
                                lhsT=xw_sb[base : base + 64,
                                           pr * PB + nt * 128 : pr * PB + nt * 128 + 128],
                                rhs=xw_sb[base : base + 64,
                                          pr * PB + NCHUNK + mh * 512 : pr * PB + NCHUNK + mh * 512 + 512],
                                start=True,
                                stop=True,
                                tile_position=(base, 0),
                            )
                        eo = j * 2048 + pr * 1024
                        nc.scalar.activation(e_b[:, eo : eo + 1024], ps[:], Exp)
                        # interleave previous-group readouts at 2-matmul
                        # granularity after each pair: long readout runs in
                        # the in-order PE queue block the next sims (which
                        # gate the exps) for up to ~1.3us during throttled
                        # phases. tile_wait_until floors them at this
                        # iteration's real-time position in the scheduler's
                        # timeline so its (optimistic, full-clock) PE model
                        # can't pack them earlier into dense runs.
                        if pending_ro:
                            cur_it = it0 - nb_g + j
                            with tc.tile_wait_until((6000 + cur_it * 2100) / 1e6):
                                emit_ro_heads(
                                    pending_ro[0], [0, 1] if pr == 0 else [2, 3]
                                )
                            if pr == 1:
                                del pending_ro[:1]

                # ---- batched softmax tail ----
                nb = len(group)
                # sp[j, x] = e[j, pairA x] + e[j, pairB x]  (x = 2 heads x 512)
                sp_b = ework.tile([128, DVE_BATCH * 1024], BF16, tag="sp")
                e3 = e_b[:, : nb * 2048].rearrange("p (j x) -> p j x", j=nb)
                nc.vector.tensor_add(
                    sp_b[:, : nb * 1024].rearrange("p (j x) -> p j x", j=nb),
                    e3[:, :, 0:1024],
                    e3[:, :, 1024:2048],
                )
                # s[j, m] = sp[j, m] + sp[j, m+512]  (fold remaining 2 heads)
                # Pool for the big steady-state groups (off the DVE), DVE for
                # the small tail groups (Pool's dispatch+contention latency
                # would sit mid-chain in the un-hidden tail).
                s_b = ework.tile([128, DVE_BATCH * 512], F32, tag="S")
                sp3 = sp_b[:, : nb * 1024].rearrange("p (j x) -> p j x", j=nb)
                add2_eng = nc.gpsimd if nb > 2 else nc.vector
                add2_eng.tensor_add(
                    s_b[:, : nb * 512].rearrange("p (j x) -> p j x", j=nb),
                    sp3[:, :, 0:512],
                    sp3[:, :, 512:1024],
                )
                # r = 1/s (fast NR approx, bf16 out at the DVE write port)
                r_b = ework.tile([128, DVE_BATCH * 512], BF16, tag="R")
                nc.vector._custom_dve(
                    _RF,
                    out=r_b[:, : nb * 512],
                    in0=s_b[:, : nb * 512],
                    s0=_RC["s0"],
                    s1=_RC["s1"],
                    imm2=_RC["imm2"],
                )
                # aff[j, h, m] = e[j, h, m] * r[j, m]
                aff_b = ework.tile([128, DVE_BATCH * 2048], BF16, tag="aff")
                if nb > 2:
                    r3 = r_b[:, : nb * 512].rearrange("p (j m) -> p j m", j=nb)
                    nc.vector.tensor_mul(
                        aff_b[:, : nb * 2048].rearrange(
                            "p (j h m) -> p j h m", j=nb, h=4
                        ),
                        e_b[:, : nb * 2048].rearrange(
                            "p (j h m) -> p j h m", j=nb, h=4
                        ),
                        r3[:, :, None, :].to_broadcast((128, nb, 4, 512)),
                    )
                else:
                    # tail: per-iteration muls so each iteration's readouts
                    # can start without waiting for the whole batch
                    for j in range(nb):
                        nc.vector.tensor_mul(
                            aff_b[:, j * 2048 : (j + 1) * 2048].rearrange(
                                "p (h m) -> p h m", h=4
                            ),
                            e_b[:, j * 2048 : (j + 1) * 2048].rearrange(
                                "p (h m) -> p h m", h=4
                            ),
                            r_b[:, j * 512 : (j + 1) * 512][
                                :, None, :
                            ].to_broadcast((128, 4, 512)),
                        )

                # queue this group's readouts; emitted interleaved with the
                # next group's sims (or drained after the loop).
                pending_ro.extend((mh, nt, aff_b, j) for j, (mh, nt) in enumerate(group))
            emit_readouts(pending_ro)
    return nc


def host_decompose(mk, qk, ms, qe, mv):
    """Build the 8 per-core input dicts."""
    mk_f = np.asarray(mk, np.float32).reshape(B, CK, THW)
    mv_f = np.asarray(mv, np.float32).reshape(B, CV, THW)
    ms_f = np.asarray(ms, np.float32).reshape(B, THW)
    qk_h = np.asarray(qk, np.float32).reshape(B, HEADS, C, HW)
    qe_h = np.asarray(qe, np.float32).reshape(B, HEADS, C, HW)

    msn = ms_f / np.float32(np.sqrt(CK))                       # [B, THW]
    mk3 = mk_f * mk_f * mk_f                                   # [B, CK, THW]

    # w [B, 33, h, m]
    w_all = np.empty((B, KDIM, HEADS, HW), np.float32)
    w_all[:, :C] = -np.swapaxes(qe_h, 1, 2)
    w_all[:, C : 2 * C] = np.swapaxes(2.0 * qk_h * qe_h, 1, 2)
    w_all[:, 2 * C] = -np.sum(qe_h * qk_h**3, axis=2)

    # x [B, 33, h, n]
    x_all = np.empty((B, KDIM, HEADS, THW), np.float32)
    mk3_h = mk3.reshape(B, HEADS, C, THW)
    mk_h = mk_f.reshape(B, HEADS, C, THW)
    x_all[:, :C] = np.swapaxes(mk3_h, 1, 2) * msn[:, None, None, :]
    x_all[:, C : 2 * C] = np.swapaxes(mk_h, 1, 2) * msn[:, None, None, :]
    x_all[:, 2 * C] = msn[:, None, :]

    xw_np = _np_dt(SIM_DT)
    PB = NCHUNK + HW
    in_maps = []
    for core in range(8):
        b, j = core // 4, core % 4
        sl = slice(j * NCHUNK, (j + 1) * NCHUNK)
        xw = np.zeros((128, 2 * PB), np.float32)
        for pr in range(2):
            for half in range(2):
                h = 2 * pr + half
                r0 = half * 64
                xw[r0 : r0 + KDIM, pr * PB : pr * PB + NCHUNK] = x_all[b, :, h, sl]
                xw[r0 : r0 + KDIM, pr * PB + NCHUNK : (pr + 1) * PB] = w_all[b, :, h]
        mvt = np.ascontiguousarray(mv_f[b, :, sl].T).astype(_BF16_NP)
        in_maps.append({"xw": xw.astype(xw_np), "mvt": mvt})
    return in_maps


_NC_CACHE = None


def _get_nc():
    global _NC_CACHE
    if _NC_CACHE is None:
        nc = build_bass()
        if not nc.is_finalized():
            nc.finalize()  # Bacc compile: wait legalization etc.
        _NC_CACHE = nc
    return _NC_CACHE


def kernel(mk, qk, ms, qe, mv, qv, _trace=False, _trace_kwargs=None):
    in_maps = host_decompose(mk, qk, ms, qe, mv)
    nc = _get_nc()
    res = run_bass_kernel_spmd(
        nc, in_maps, list(range(8)), trace=_trace, **(_trace_kwargs or {})
    )
    mem = np.zeros((B, CV, HW), np.float32)
    for core in range(8):
        mem[core // 4] += res.results[core]["mem"]
    out = np.concatenate(
        [mem.reshape(B, CV, H, W), np.asarray(qv, np.float32).reshape(B, CV, H, W)],
        axis=1,
    )
    if _trace:
        return out, res
    return out
